# revision 11
# baseline (speedup 1.0000x reference)
"""Trainium2 Bass kernel for CapsuleLayer dynamic routing.

Problem: x [512, 1152, 8], W [1152, 10, 16, 8] -> v [512, 10, 16, 1]
  pred[b,p,n,t] = sum_d W[p,n,t,d] x[b,p,d]
  3 routing iterations; the b_ij update adds a batch-mean (keepdim) term, so
  b_ij is constant across batch => coupling coeffs are c[p,n] shared by all b.

Strategy: shard P across the 8 cores (144 prev-caps each). pred is never
materialized; c[p,n] is folded into W (Wc = W * c broadcast) so
  s[b,nt] = sum_pd x[b,pd] Wc[pd,nt]        (PE, contraction over local pd)
and the agreement batch-mean uses
  M[pd,nt] = (1/B) sum_b x[b,pd] v[b,nt]    (PE)
  abar[p,n] = sum_{d,t} W2[pd,nt] M[pd,nt]  (DVE mul + group reduce + S-matmul)
Cross-core: AllReduce of partial s for iters 0-1; ReduceScatter for the last
iteration (each core squashes + outputs its partition slice; host assembles).

GEMM_DT selects the matmul operand dtype: "f32" (exact, dual-pass PE),
"f32r" (single-pass, 11-bit mantissa), "bf16" (single-pass, 8-bit).
"""

import os
import sys

sys.path.insert(0, "/opt/trn_rl_repo")

import numpy as np

import concourse.bacc as bacc
import concourse.bass as bass
import concourse.mybir as mybir
import concourse.tile as tile
from concourse.bass_utils import run_bass_kernel_spmd

F32 = mybir.dt.float32
AF = mybir.ActivationFunctionType
ALU = mybir.AluOpType

B, P, N, T, D = 512, 1152, 10, 16, 8
NCORES = 8
PLOC = P // NCORES          # 144 prev caps per core
PD = PLOC * D               # 1152 contraction length per core
CH = PD // 128              # 9 chunks of 128 partitions
BB = B // 128               # 4 batch blocks
NT = N * T                  # 160
NITER = 3

GEMM_DT = os.environ.get("CAPS_GEMM_DT", "f32")

_CACHE = {}


def _dt():
    return {"f32": F32, "f32r": mybir.dt.float32r,
            "bf16": mybir.dt.bfloat16}[GEMM_DT]


def _build():
    if "nc" in _CACHE:
        return _CACHE["nc"]

    DT = _dt()
    nc = bacc.Bacc("TRN2", target_bir_lowering=False, debug=False,
                   num_devices=NCORES)

    x2_d = nc.dram_tensor("x2", [128, BB, PD], DT, kind="ExternalInput")
    x2t_d = nc.dram_tensor("x2t", [128, CH, B], DT, kind="ExternalInput")
    w2g_d = nc.dram_tensor("w2g", [128, CH, NT], DT, kind="ExternalInput")
    w2f_d = nc.dram_tensor("w2f", [128, CH, NT], F32, kind="ExternalInput")
    smat_d = nc.dram_tensor("smat", [128, 16], F32, kind="ExternalInput")
    stmat_d = nc.dram_tensor("stmat", [16, 128], F32, kind="ExternalInput")
    vout_d = nc.dram_tensor("vout", [16, BB, NT], F32, kind="ExternalOutput")

    rg = [list(range(NCORES))]

    def squash(nc, wpool, s_full, lam, parts, tagp, out_dt):
        """v = lam*s*f with f = sq/(1+sq*norm); returns v tile [parts,BB,NT]."""
        s2 = wpool.tile([parts, BB, NT], F32, tag="s2" + tagp)
        nc.vector.tensor_tensor(s2[:], s_full[:], s_full[:], ALU.mult)
        sqr = wpool.tile([parts, BB * N], F32, tag="sqr" + tagp)
        nc.vector.tensor_reduce(
            sqr[:], s2.rearrange("p a (n t) -> p (a n) t", t=T),
            axis=mybir.AxisListType.X, op=ALU.add)
        norm = wpool.tile([parts, BB * N], F32, tag="norm" + tagp)
        nc.scalar.activation(norm[:], sqr[:], AF.Sqrt, scale=lam * lam)
        sq = wpool.tile([parts, BB * N], F32, tag="sq" + tagp)
        nc.vector.tensor_scalar_mul(sq[:], sqr[:], lam * lam)
        den = wpool.tile([parts, BB * N], F32, tag="den" + tagp)
        nc.vector.tensor_tensor(den[:], sq[:], norm[:], ALU.mult)
        nc.vector.tensor_scalar_add(den[:], den[:], 1.0)
        rden = wpool.tile([parts, BB * N], F32, tag="rden" + tagp)
        nc.vector.reciprocal(rden[:], den[:])
        fmul = wpool.tile([parts, BB * N], F32, tag="fmul" + tagp)
        nc.vector.tensor_tensor(fmul[:], sq[:], rden[:], ALU.mult)
        nc.vector.tensor_scalar_mul(fmul[:], fmul[:], lam)
        v = wpool.tile([parts, BB, NT], out_dt, tag="v" + tagp)
        nc.vector.tensor_tensor(
            v.rearrange("p a (n t) -> p a n t", t=T),
            s_full.rearrange("p a (n t) -> p a n t", t=T),
            fmul.rearrange("p (a n) -> p a n", n=N)
                .unsqueeze(3).broadcast_to([parts, BB, N, T]),
            ALU.mult)
        return v

    with tile.TileContext(nc) as tc:
        with (
            tc.tile_pool(name="const", bufs=1) as cpool,
            tc.tile_pool(name="work", bufs=2) as wpool,
            tc.tile_pool(name="ps_s", bufs=4, space="PSUM") as ps_s,
            tc.tile_pool(name="ps_m", bufs=2, space="PSUM") as ps_m,
            tc.tile_pool(name="dram", bufs=2, space="DRAM") as dpool,
        ):
            # chunked loads so iter-0 matmuls overlap the input DMA
            x2t = []
            w2g = []
            for c in range(CH):
                xt = cpool.tile([128, B], DT, tag=f"x2t{c}")
                nc.sync.dma_start(xt[:], x2t_d[:, c, :])
                x2t.append(xt)
                wg = cpool.tile([128, NT], DT, tag=f"w2g{c}")
                nc.sync.dma_start(wg[:], w2g_d[:, c, :])
                w2g.append(wg)
            smat = cpool.tile([128, 16], F32)
            nc.sync.dma_start(smat[:], smat_d[:])
            stmat = cpool.tile([16, 128], F32)
            nc.sync.dma_start(stmat[:], stmat_d[:])
            w2f = cpool.tile([128, CH, NT], F32)
            nc.sync.dma_start(w2f[:], w2f_d[:])
            x2 = cpool.tile([128, BB, PD], DT)
            nc.sync.dma_start(x2[:], x2_d[:])

            wc = w2g            # iteration 0: uniform c folded via lam=1/N
            lam = 1.0 / N
            bbar = None

            for it in range(NITER):
                # ---- s partial: s[b_blk, nt] += x2t_c[:,blk].T @ wc_c
                s_sb = wpool.tile([128, BB, NT], F32, tag="s_sb")
                for bb in range(BB):
                    s_ps = ps_s.tile([128, NT], F32, tag="s_ps")
                    for c in range(CH):
                        nc.tensor.matmul(
                            s_ps[:], x2t[c][:, bb * 128:(bb + 1) * 128],
                            wc[c][:], start=(c == 0), stop=(c == CH - 1))
                    nc.vector.tensor_copy(s_sb[:, bb, :], s_ps[:])

                # split the bounce DMA across queues (one dma -> one queue)
                cc_in = dpool.tile([128, BB, NT], F32, tag="cc_in")
                for bb in range(BB):
                    for h in range(2):
                        sl = slice(h * (NT // 2), (h + 1) * (NT // 2))
                        nc.sync.dma_start(cc_in[:, bb, sl], s_sb[:, bb, sl])

                if it == NITER - 1:
                    # ---- final: ReduceScatter; each core outputs its slice
                    cc_rs = dpool.tile([16, BB, NT], F32, tag="cc_rs")
                    nc.gpsimd.collective_compute(
                        "ReduceScatter", ALU.add, replica_groups=rg,
                        ins=[cc_in.opt()], outs=[cc_rs.opt()])
                    s_last = wpool.tile([16, BB, NT], F32, tag="s_last")
                    nc.sync.dma_start(s_last[:], cc_rs[:])
                    v = squash(nc, wpool, s_last, lam, 16, "L", F32)
                    nc.sync.dma_start(vout_d[:], v[:])
                    break

                # ---- AllReduce partial s over the 8 P-shards
                cc_out = dpool.tile([128, BB, NT], F32, tag="cc_out")
                nc.gpsimd.collective_compute(
                    "AllReduce", ALU.add, replica_groups=rg,
                    ins=[cc_in.opt()], outs=[cc_out.opt()])
                s_full = wpool.tile([128, BB, NT], F32, tag="s_full")
                for bb in range(BB):
                    for h in range(2):
                        sl = slice(h * (NT // 2), (h + 1) * (NT // 2))
                        nc.sync.dma_start(s_full[:, bb, sl], cc_out[:, bb, sl])

                v_g = squash(nc, wpool, s_full, lam, 128, "", DT)

                # ---- routing update
                # M[pd, nt] = sum_b x2[b, pd] v[b, nt]   (1/B folded in smat)
                rtile = wpool.tile([128, CH * N], F32, tag="rtile")
                for c in range(CH):
                    m_ps = ps_m.tile([128, NT], F32, tag="m_ps")
                    for bb in range(BB):
                        nc.tensor.matmul(
                            m_ps[:], x2[:, bb, c * 128:(c + 1) * 128],
                            v_g[:, bb, :], start=(bb == 0), stop=(bb == BB - 1))
                    e_sb = wpool.tile([128, NT], F32, tag="e_sb", bufs=3)
                    nc.vector.tensor_tensor(
                        e_sb[:], w2f[:, c, :], m_ps[:], ALU.mult)
                    nc.vector.tensor_reduce(
                        rtile[:, c * N:(c + 1) * N],
                        e_sb.rearrange("p (n t) -> p n t", t=T),
                        axis=mybir.AxisListType.X, op=ALU.add)

                # abar[pl, (c,n)] = sum_d R[(pl,d), (c,n)] / B   via smat
                a_ps = ps_m.tile([16, CH * N], F32, tag="a_ps", bufs=1)
                nc.tensor.matmul(a_ps[:], smat[:], rtile[:],
                                 start=True, stop=True)

                bnew = wpool.tile([16, CH * N], F32, tag="bbar")
                if bbar is None:
                    nc.vector.tensor_copy(bnew[:], a_ps[:])
                else:
                    nc.vector.tensor_tensor(bnew[:], bbar[:], a_ps[:], ALU.add)
                bbar = bnew

                # softmax over n (innermost groups of N)
                eb = wpool.tile([16, CH * N], F32, tag="eb")
                nc.scalar.activation(eb[:], bbar[:], AF.Exp)
                ssum = wpool.tile([16, CH], F32, tag="ssum")
                nc.vector.tensor_reduce(
                    ssum[:], eb.rearrange("p (c n) -> p c n", n=N),
                    axis=mybir.AxisListType.X, op=ALU.add)
                rsum = wpool.tile([16, CH], F32, tag="rsum")
                nc.vector.reciprocal(rsum[:], ssum[:])
                cb16 = wpool.tile([16, CH * N], F32, tag="cb16")
                nc.vector.tensor_tensor(
                    cb16.rearrange("p (c n) -> p c n", n=N),
                    eb.rearrange("p (c n) -> p c n", n=N),
                    rsum.unsqueeze(2).broadcast_to([16, CH, N]),
                    ALU.mult)

                # broadcast c over d: cb[(pl,d), (c,n)] via stmat
                cb_ps = ps_m.tile([128, CH * N], F32, tag="cb_ps", bufs=1)
                nc.tensor.matmul(cb_ps[:], stmat[:], cb16[:],
                                 start=True, stop=True)
                cb = wpool.tile([128, CH * N], F32, tag="cb")
                nc.vector.tensor_copy(cb[:], cb_ps[:])

                # Wc_c = W2_c * c (broadcast over t)
                wc_new = []
                for c in range(CH):
                    wct = wpool.tile([128, NT], DT, tag=f"wct{c}")
                    nc.vector.tensor_tensor(
                        wct.rearrange("p (n t) -> p n t", t=T),
                        w2f[:, c, :].rearrange("p (n t) -> p n t", t=T),
                        cb[:, c * N:(c + 1) * N]
                            .unsqueeze(2).broadcast_to([128, N, T]),
                        ALU.mult)
                    wc_new.append(wct)
                wc = wc_new
                lam = 1.0

    nc.compile()
    _CACHE["nc"] = nc
    return nc


def _round_f32r(a):
    # round-to-nearest-even keeping 11 mantissa bits (top 20 bits of fp32)
    u = np.ascontiguousarray(a, dtype=np.float32).view(np.uint32)
    keep = np.uint32(0xFFFFF000)
    bit = (u >> np.uint32(12)) & np.uint32(1)
    return ((u + np.uint32(0x7FF) + bit) & keep).view(np.float32)


def _cast(a):
    if GEMM_DT == "f32":
        return np.ascontiguousarray(a, dtype=np.float32)
    if GEMM_DT == "f32r":
        return _round_f32r(np.ascontiguousarray(a, dtype=np.float32))
    import ml_dtypes
    return np.ascontiguousarray(a).astype(ml_dtypes.bfloat16)


def _prep_inputs(x, W):
    x = np.ascontiguousarray(x, dtype=np.float32)
    W = np.ascontiguousarray(W, dtype=np.float32)
    # smat[pl*8+d, pl] = 1/B ; stmat[pl, pl*8+d] = 1
    smat = np.kron(np.eye(16, dtype=np.float32),
                   np.ones((D, 1), np.float32)) / float(B)   # [128, 16]
    stmat = np.kron(np.eye(16, dtype=np.float32),
                    np.ones((1, D), np.float32))             # [16, 128]
    in_maps = []
    for k in range(NCORES):
        ps = slice(k * PLOC, (k + 1) * PLOC)
        xk = x[:, ps, :].reshape(B, PD)                       # [b, pd]
        x2 = np.ascontiguousarray(
            xk.reshape(BB, 128, PD).transpose(1, 0, 2))       # [128, BB, PD]
        x2t = np.ascontiguousarray(
            xk.T.reshape(CH, 128, B).transpose(1, 0, 2))      # [128, CH, B]
        w2 = np.ascontiguousarray(
            W[ps].transpose(0, 3, 1, 2).reshape(CH, 128, NT).transpose(1, 0, 2))
        in_maps.append({
            "x2": _cast(x2), "x2t": _cast(x2t), "w2g": _cast(w2),
            "w2f": w2, "smat": smat, "stmat": stmat,
        })
    return in_maps


def run(x, W, trace=False):
    nc = _build()
    in_maps = _prep_inputs(x, W)
    res = run_bass_kernel_spmd(nc, in_maps, list(range(NCORES)), trace=trace)
    # each core k returns the summed partition slice [16k:16k+16] of
    # [128, BB, NT]; assemble, then b = bb*128 + p
    vfull = np.empty((128, BB, NT), dtype=np.float32)
    for k in range(NCORES):
        vfull[16 * k:16 * (k + 1)] = res.results[k]["vout"]
    v = vfull.transpose(1, 0, 2).reshape(B, N, T)
    out = np.ascontiguousarray(v[..., None], dtype=np.float32)
    return out, res.exec_time_ns


def kernel(x, W):
    return run(x, W, trace=False)[0]
